# revision 8
# baseline (speedup 1.0000x reference)
"""Trainium2 Bass kernel for BilinearSeqAttnAction:

    w = weight[actions]              # [B, Y, X]
    Wy = einsum('by,byx->bx', y, w) + bias[actions]
    xWy = einsum('blx,bx->bl', x, Wy)
    alpha = log_softmax(where(x_mask, -inf, xWy), axis=-1)

Sharding (8 NeuronCores):
  Stage 1 (Wy) is sharded BY ACTION: core i owns weight[4i:4i+4] (8MB bf16,
  read exactly once fleet-wide) and computes a masked partial Wy for all
  128 batches using host-zeroed yT columns; a 256KB bf16 ReduceScatter then
  hands core i the finished Wy rows for batches 16i:16i+16.  (Each Wy row
  has exactly one nonzero contributor across cores, so bf16 partials add
  no accumulation error beyond the bf16 cast the lhsT needs anyway.)
  Stage 2 (xWy + log_softmax) is sharded BY BATCH: core i streams its 16
  x[b] slices (host-transposed to [X, L] so the X-contraction runs on the
  TensorEngine in single-pass bf16) against WyT columns; scores accumulate
  [16, L] f32 in PSUM with batch on partitions, so the softmax tail is
  per-partition free-dim work.  Big streaming DMAs (x, w, ytm) ride the SP
  HWDGE ring; small control DMAs ride the ACT ring to keep issue latency
  off the streaming path.
"""

import numpy as np

from ml_dtypes import bfloat16 as _np_bf16

N_CORES = 8
B, L, X, Y, A = 128, 1024, 1024, 1024, 32
BC = B // N_CORES  # batches per core
AC = A // N_CORES  # actions per core
CH = X // 128      # 128-wide chunks of the contraction dim
P = 128

_cached = {}


def _build_program():
    from concourse import bacc, tile, mybir
    from concourse.tile import add_dep_helper

    f32 = mybir.dt.float32
    bf16 = mybir.dt.bfloat16
    u8 = mybir.dt.uint8
    AF = mybir.ActivationFunctionType

    nc = bacc.Bacc(
        "TRN2",
        target_bir_lowering=False,
        debug=False,
        enable_asserts=False,
        num_devices=N_CORES,
    )

    xt_d = nc.dram_tensor("xt", [BC, P, CH * L], bf16, kind="ExternalInput").ap()
    ytm_d = nc.dram_tensor("ytm", [P, AC * CH * B], bf16, kind="ExternalInput").ap()
    wt_d = nc.dram_tensor("wt", [AC, P, CH * X], bf16, kind="ExternalInput").ap()
    biasg_d = nc.dram_tensor("biasg", [BC, X], f32, kind="ExternalInput").ap()
    mask_d = nc.dram_tensor("masku", [BC, L], u8, kind="ExternalInput").ap()
    eye_d = nc.dram_tensor("eye16", [BC, BC], f32, kind="ExternalInput").ap()
    out_d = nc.dram_tensor("out", [BC, L], f32, kind="ExternalOutput").ap()

    with tile.TileContext(nc) as tc:
        with (
            tc.tile_pool(name="dram", bufs=1, space="DRAM") as dram,
            tc.tile_pool(name="ypool", bufs=1) as ypool,
            tc.tile_pool(name="wpool", bufs=3) as wpool,
            tc.tile_pool(name="tmppool", bufs=2) as tmppool,
            tc.tile_pool(name="xpool", bufs=6) as xpool,
            tc.tile_pool(name="small", bufs=1) as small,
            tc.tile_pool(name="ps_wy", bufs=1, space="PSUM") as ps_wy_pool,
            tc.tile_pool(name="ps_t", bufs=1, space="PSUM") as ps_t_pool,
            tc.tile_pool(name="ps_s", bufs=2, space="PSUM") as ps_s_pool,
        ):
            # ---- Stage 1 streaming inputs: one DMA for all of ytm, one
            # 2MB DMA per action for the weights.  These are issued first on
            # the SP ring so stage 1 is not stuck behind x transfers.
            yt_all = ypool.tile([P, AC * CH * B], bf16)
            nc.sync.dma_start(yt_all[:], ytm_d[:])
            w_tiles = []
            w_last_dma = None
            for a in range(AC):
                w_t = wpool.tile([P, CH * X], bf16, name="w_t", tag="w_t")
                w_last_dma = nc.sync.dma_start(w_t[:], wt_d[a])
                w_tiles.append(w_t)

            ps_wy = ps_wy_pool.tile([P, X], f32)
            for a in range(AC):
                for k in range(CH):
                    lhs = yt_all[:, (a * CH + k) * B : (a * CH + k + 1) * B]
                    for j in range(2):
                        nc.tensor.matmul(
                            ps_wy[:, j * 512 : (j + 1) * 512],
                            lhs,
                            w_tiles[a][:, k * X + j * 512 : k * X + (j + 1) * 512],
                            start=(a == 0 and k == 0),
                            stop=(a == AC - 1 and k == CH - 1),
                        )

            wy_part = small.tile([P, X], bf16)
            nc.scalar.copy(wy_part[:], ps_wy[:])
            cin = dram.tile([P, X], bf16)
            cout = dram.tile([BC, X], bf16)
            cin_dma = nc.scalar.dma_start(cin[:], wy_part[:])
            nc.gpsimd.collective_compute(
                "ReduceScatter",
                mybir.AluOpType.add,
                replica_groups=[list(range(N_CORES))],
                ins=[cin[:]],
                outs=[cout[:]],
            )
            wy_bf = small.tile([BC, X], bf16)
            nc.scalar.dma_start(wy_bf[:], cout[:])

            bias_sb = small.tile([BC, X], f32)
            nc.scalar.dma_start(bias_sb[:], biasg_d[:])
            wy_sb = small.tile([BC, X], f32)
            nc.vector.tensor_add(wy_sb[:], wy_bf[:], bias_sb[:])

            # ---- Transpose Wy [16, X] -> WyT [X-chunk partitions, 16] so it
            # can be the stationary matmul operand of stage 2 (cast to bf16).
            eye_sb = small.tile([BC, BC], f32)
            nc.scalar.dma_start(eye_sb[:], eye_d[:])
            warm_in = small.tile([BC, 1], f32)
            nc.vector.memset(warm_in[:], 1.0)
            warm_out = small.tile([BC, 1], f32)
            nc.scalar.activation(warm_out[:], warm_in[:], AF.Exp)
            nc.scalar.activation(warm_out[:], warm_in[:], AF.Ln)
            ps_t = ps_t_pool.tile([P, CH * BC], f32)
            for c in range(CH):
                nc.tensor.transpose(
                    ps_t[:, c * BC : (c + 1) * BC],
                    wy_sb[:, c * P : (c + 1) * P],
                    eye_sb[:],
                )
            wyT = small.tile([P, CH * BC], bf16)
            nc.scalar.copy(wyT[:], ps_t[:])

            # ---- Stage 2: scores[b, l] = sum_x xT[b][x, l] * Wy[b, x].
            # lhsT holds ALL 16 Wy columns, so each matmul produces the full
            # 16xL cross-product; only row b is the real batch-b result.
            # Compute engines can't address single partitions (32-aligned
            # base rule), so copy the whole block to SBUF and let a DMA
            # gather row b into the scores tile.
            scores = small.tile([BC, L], f32)
            mask_sb = small.tile([BC, L], u8)
            nc.scalar.dma_start(mask_sb[:], mask_d[:])
            neg_sb = small.tile([BC, L], f32)
            nc.vector.memset(neg_sb[:], float("-inf"))
            for b in range(BC):
                xb = xpool.tile([P, CH * L], bf16, name="xb", tag="xb")
                x_dma = nc.sync.dma_start(xb[:], xt_d[b])
                add_dep_helper(
                    x_dma.ins,
                    cin_dma.ins,
                    sync=True,
                    reason="x stream yields HBM bandwidth to the Wy critical path",
                )
                ps_s = ps_s_pool.tile([BC, L], f32, name="ps_s", tag="ps_s")
                for c in range(CH):
                    for j in range(2):
                        nc.tensor.matmul(
                            ps_s[:, j * 512 : (j + 1) * 512],
                            wyT[:, c * BC : (c + 1) * BC],
                            xb[:, c * L + j * 512 : c * L + (j + 1) * 512],
                            start=(c == 0),
                            stop=(c == CH - 1),
                        )
                tmp = tmppool.tile([BC, L], f32, name="tmp", tag="tmp")
                nc.scalar.copy(tmp[:], ps_s[:])
                nc.vector.copy_predicated(tmp[:], mask_sb[:], neg_sb[:])
                nc.scalar.dma_start(scores[b : b + 1, :], tmp[b : b + 1, :])

            # ---- log_softmax, batch on partitions throughout.
            negm = small.tile([BC, 1], f32)
            nc.vector.reduce_max(
                negm[:], scores[:], axis=mybir.AxisListType.X, negate=True
            )
            scratch = small.tile([BC, L], f32)
            ssum = small.tile([BC, 1], f32)
            nc.scalar.activation(
                scratch[:], scores[:], AF.Exp, bias=negm[:], scale=1.0, accum_out=ssum[:]
            )
            lg = small.tile([BC, 1], f32)
            nc.scalar.activation(lg[:], ssum[:], AF.Ln)
            shift = small.tile([BC, 1], f32)
            nc.vector.tensor_sub(shift[:], negm[:], lg[:])
            nc.vector.tensor_scalar_add(scratch[:], scores[:], shift[:])
            nc.scalar.dma_start(out_d[:], scratch[:])

    nc.compile()
    return nc


def _get_program():
    if "nc" not in _cached:
        _cached["nc"] = _build_program()
    return _cached["nc"]


def kernel(**inputs) -> np.ndarray:
    x = np.asarray(inputs["x"], dtype=np.float32)
    y = np.asarray(inputs["y"], dtype=np.float32)
    x_mask = np.asarray(inputs["x_mask"])
    actions = np.asarray(inputs["actions"]).astype(np.int64)
    weight = np.asarray(inputs["weight"], dtype=np.float32)
    bias = np.asarray(inputs["bias"], dtype=np.float32)

    nc = _get_program()
    eye = np.eye(BC, dtype=np.float32)

    in_maps = []
    for i in range(N_CORES):
        sl = slice(BC * i, BC * (i + 1))
        # Single-stride device layouts: [..., p, k, inner] so each DMA is a
        # plain 2D transfer with 16KB contiguous partition lines.
        xt = np.ascontiguousarray(
            x[sl].reshape(BC, L, CH, P).transpose(0, 3, 2, 1)
        ).astype(_np_bf16).reshape(BC, P, CH * L)
        ytm = np.zeros((AC, Y, B), dtype=np.float32)
        for j in range(AC):
            sel = actions == (AC * i + j)
            if sel.any():
                ytm[j][:, sel] = y[sel].T
        ytm = np.ascontiguousarray(
            ytm.reshape(AC, CH, P, B).transpose(2, 0, 1, 3)
        ).reshape(P, AC * CH * B)
        in_maps.append(
            {
                "xt": xt,
                "ytm": ytm.astype(_np_bf16),
                "wt": np.ascontiguousarray(
                    weight[AC * i : AC * (i + 1)]
                    .reshape(AC, CH, P, X)
                    .transpose(0, 2, 1, 3)
                ).astype(_np_bf16).reshape(AC, P, CH * X),
                "biasg": np.ascontiguousarray(bias[actions[sl]]),
                "masku": x_mask[sl].astype(np.uint8),
                "eye16": eye,
            }
        )

    from concourse import bass_utils

    res = bass_utils.run_bass_kernel_spmd(
        nc, in_maps, core_ids=list(range(N_CORES))
    )
    _cached["last_results"] = res
    return np.concatenate([res.results[i]["out"] for i in range(N_CORES)], axis=0)


# revision 9
# speedup vs baseline: 1.7594x; 1.7594x over previous
"""Trainium2 Bass kernel for BilinearSeqAttnAction:

    w = weight[actions]              # [B, Y, X]
    Wy = einsum('by,byx->bx', y, w) + bias[actions]
    xWy = einsum('blx,bx->bl', x, Wy)
    alpha = log_softmax(where(x_mask, -inf, xWy), axis=-1)

Sharding (8 NeuronCores, fully data-parallel, no collectives):
  The host packs batches onto cores grouped by action (a batch's slot
  assignment is pure indexing), so each core's 16 batches span only ~4-5
  distinct actions.  Each core loads just those dedup'd weight matrices
  (G slots of 2MB bf16; G = max distinct over cores, same program on all
  cores) and computes Wy for its own batches with masked yT columns:
  slot g's lhsT holds y[b] in column lb only if batch lb uses slot g's
  action, so accumulating all slots over the contraction dim yields each
  batch's own y @ weight[action].  Stage 2 streams the core's 16
  host-transposed x[b] slices ([X, L] so the X-contraction runs on the
  TensorEngine in single-pass bf16) against WyT columns; scores
  accumulate [16, L] f32 in PSUM with batch on partitions, so the
  log_softmax tail is per-partition free-dim work.  The x stream is
  sync-gated behind the last weight DMA so stage 1 gets the full HBM
  bandwidth up front; outputs are unsorted on the host.
"""

import numpy as np

from ml_dtypes import bfloat16 as _np_bf16

N_CORES = 8
B, L, X, Y, A = 128, 1024, 1024, 1024, 32
BC = B // N_CORES  # batches per core
CH = X // 128      # 128-wide chunks of the contraction dim
P = 128

_cached = {}


def _build_program(G):
    from concourse import bacc, tile, mybir
    from concourse.tile import add_dep_helper

    f32 = mybir.dt.float32
    bf16 = mybir.dt.bfloat16
    u8 = mybir.dt.uint8
    AF = mybir.ActivationFunctionType

    nc = bacc.Bacc(
        "TRN2",
        target_bir_lowering=False,
        debug=False,
        enable_asserts=False,
        num_devices=N_CORES,
    )

    xt_d = nc.dram_tensor("xt", [BC, P, CH * L], bf16, kind="ExternalInput").ap()
    ytm_d = nc.dram_tensor("ytm", [P, G * CH * BC], bf16, kind="ExternalInput").ap()
    wt_d = nc.dram_tensor("wt", [G, P, CH * X], bf16, kind="ExternalInput").ap()
    biasg_d = nc.dram_tensor("biasg", [BC, X], f32, kind="ExternalInput").ap()
    mask_d = nc.dram_tensor("masku", [BC, L], u8, kind="ExternalInput").ap()
    eye_d = nc.dram_tensor("eye16", [BC, BC], f32, kind="ExternalInput").ap()
    out_d = nc.dram_tensor("out", [BC, L], f32, kind="ExternalOutput").ap()

    with tile.TileContext(nc) as tc:
        with (
            tc.tile_pool(name="ypool", bufs=1) as ypool,
            tc.tile_pool(name="wpool", bufs=3) as wpool,
            tc.tile_pool(name="tmppool", bufs=2) as tmppool,
            tc.tile_pool(name="xpool", bufs=6) as xpool,
            tc.tile_pool(name="small", bufs=1) as small,
            tc.tile_pool(name="ps_wy", bufs=1, space="PSUM") as ps_wy_pool,
            tc.tile_pool(name="ps_t", bufs=1, space="PSUM") as ps_t_pool,
            tc.tile_pool(name="ps_s", bufs=2, space="PSUM") as ps_s_pool,
        ):
            # ---- Stage 1: Wy for this core's own 16 batches over its G
            # dedup'd weight slots.  One DMA for all of ytm, one 2MB DMA per
            # slot, all issued first on the SP ring.
            yt_all = ypool.tile([P, G * CH * BC], bf16)
            nc.sync.dma_start(yt_all[:], ytm_d[:])
            w_tiles = []
            w_last_dma = None
            for g in range(G):
                w_t = wpool.tile([P, CH * X], bf16, name="w_t", tag="w_t")
                w_last_dma = nc.sync.dma_start(w_t[:], wt_d[g])
                w_tiles.append(w_t)

            ps_wy = ps_wy_pool.tile([BC, X], f32)
            for g in range(G):
                for k in range(CH):
                    lhs = yt_all[:, (g * CH + k) * BC : (g * CH + k + 1) * BC]
                    for j in range(2):
                        nc.tensor.matmul(
                            ps_wy[:, j * 512 : (j + 1) * 512],
                            lhs,
                            w_tiles[g][:, k * X + j * 512 : k * X + (j + 1) * 512],
                            start=(g == 0 and k == 0),
                            stop=(g == G - 1 and k == CH - 1),
                        )

            bias_sb = small.tile([BC, X], f32)
            nc.scalar.dma_start(bias_sb[:], biasg_d[:])
            wy_sb = small.tile([BC, X], f32)
            nc.vector.tensor_add(wy_sb[:], ps_wy[:], bias_sb[:])

            # ---- Transpose Wy [16, X] -> WyT [X-chunk partitions, 16] so it
            # can be the stationary matmul operand of stage 2 (cast to bf16).
            eye_sb = small.tile([BC, BC], f32)
            nc.scalar.dma_start(eye_sb[:], eye_d[:])
            warm_in = small.tile([BC, 1], f32)
            nc.vector.memset(warm_in[:], 1.0)
            warm_out = small.tile([BC, 1], f32)
            nc.scalar.activation(warm_out[:], warm_in[:], AF.Exp)
            nc.scalar.activation(warm_out[:], warm_in[:], AF.Ln)
            ps_t = ps_t_pool.tile([P, CH * BC], f32)
            for c in range(CH):
                nc.tensor.transpose(
                    ps_t[:, c * BC : (c + 1) * BC],
                    wy_sb[:, c * P : (c + 1) * P],
                    eye_sb[:],
                )
            wyT = small.tile([P, CH * BC], bf16)
            nc.scalar.copy(wyT[:], ps_t[:])

            # ---- Stage 2: scores[b, l] = sum_x xT[b][x, l] * Wy[b, x].
            # lhsT holds ALL 16 Wy columns, so each matmul produces the full
            # 16xL cross-product; only row b is the real batch-b result.
            # Compute engines can't address single partitions (32-aligned
            # base rule), so copy the whole block to SBUF and let a DMA
            # gather row b into the scores tile.
            scores = small.tile([BC, L], f32)
            mask_sb = small.tile([BC, L], u8)
            nc.scalar.dma_start(mask_sb[:], mask_d[:])
            neg_sb = small.tile([BC, L], f32)
            nc.vector.memset(neg_sb[:], float("-inf"))
            for b in range(BC):
                xb = xpool.tile([P, CH * L], bf16, name="xb", tag="xb")
                x_dma = nc.sync.dma_start(xb[:], xt_d[b])
                add_dep_helper(
                    x_dma.ins,
                    w_last_dma.ins,
                    sync=True,
                    reason="x stream yields HBM bandwidth to stage-1 weights",
                )
                ps_s = ps_s_pool.tile([BC, L], f32, name="ps_s", tag="ps_s")
                for c in range(CH):
                    for j in range(2):
                        nc.tensor.matmul(
                            ps_s[:, j * 512 : (j + 1) * 512],
                            wyT[:, c * BC : (c + 1) * BC],
                            xb[:, c * L + j * 512 : c * L + (j + 1) * 512],
                            start=(c == 0),
                            stop=(c == CH - 1),
                        )
                tmp = tmppool.tile([BC, L], f32, name="tmp", tag="tmp")
                nc.scalar.copy(tmp[:], ps_s[:])
                nc.vector.copy_predicated(tmp[:], mask_sb[:], neg_sb[:])
                nc.scalar.dma_start(scores[b : b + 1, :], tmp[b : b + 1, :])

            # ---- log_softmax, batch on partitions throughout.
            negm = small.tile([BC, 1], f32)
            nc.vector.reduce_max(
                negm[:], scores[:], axis=mybir.AxisListType.X, negate=True
            )
            scratch = small.tile([BC, L], f32)
            ssum = small.tile([BC, 1], f32)
            nc.scalar.activation(
                scratch[:], scores[:], AF.Exp, bias=negm[:], scale=1.0, accum_out=ssum[:]
            )
            lg = small.tile([BC, 1], f32)
            nc.scalar.activation(lg[:], ssum[:], AF.Ln)
            shift = small.tile([BC, 1], f32)
            nc.vector.tensor_sub(shift[:], negm[:], lg[:])
            nc.vector.tensor_scalar_add(scratch[:], scores[:], shift[:])
            nc.scalar.dma_start(out_d[:], scratch[:])

    nc.compile()
    return nc


def _get_program(G):
    key = ("nc", G)
    if key not in _cached:
        _cached[key] = _build_program(G)
    return _cached[key]


def _pack_batches(actions):
    """Assign batches to cores grouped by action so each core sees few
    distinct actions.  Greedy largest-group-first bin packing with group
    splitting; returns (order, slots) where order is the batch permutation
    (16 per core) and slots[i] is core i's list of (action, local batch
    indices) weight slots."""
    groups = {}
    for b, a in enumerate(actions.tolist()):
        groups.setdefault(a, []).append(b)
    order_groups = sorted(groups.items(), key=lambda kv: -len(kv[1]))
    free = [BC] * N_CORES
    slots = [[] for _ in range(N_CORES)]
    for a, bs in order_groups:
        rest = bs
        while rest:
            i = max(range(N_CORES), key=lambda c: free[c])
            take = min(free[i], len(rest))
            slots[i].append((a, rest[:take]))
            free[i] -= take
            rest = rest[take:]
    order = []
    for i in range(N_CORES):
        for a, bs in slots[i]:
            order.extend(bs)
    return np.array(order), slots


def kernel(**inputs) -> np.ndarray:
    x = np.asarray(inputs["x"], dtype=np.float32)
    y = np.asarray(inputs["y"], dtype=np.float32)
    x_mask = np.asarray(inputs["x_mask"])
    actions = np.asarray(inputs["actions"]).astype(np.int64)
    weight = np.asarray(inputs["weight"], dtype=np.float32)
    bias = np.asarray(inputs["bias"], dtype=np.float32)

    order, slots = _pack_batches(actions)
    G = max(len(s) for s in slots)
    nc = _get_program(G)
    eye = np.eye(BC, dtype=np.float32)
    wbf = weight.reshape(A, CH, P, X).transpose(0, 2, 1, 3).astype(_np_bf16)

    in_maps = []
    for i in range(N_CORES):
        sel = order[BC * i : BC * (i + 1)]
        # Single-stride device layouts: [..., p, k, inner] so each DMA is a
        # plain 2D transfer with 16KB contiguous partition lines.
        xt = np.ascontiguousarray(
            x[sel].reshape(BC, L, CH, P).transpose(0, 3, 2, 1)
        ).astype(_np_bf16).reshape(BC, P, CH * L)
        ytm = np.zeros((G, Y, BC), dtype=np.float32)
        wt = np.zeros((G, P, CH * X), dtype=_np_bf16)
        base = 0
        for g, (a, bs) in enumerate(slots[i]):
            lbs = list(range(base, base + len(bs)))
            base += len(bs)
            ytm[g][:, lbs] = y[bs].T
            wt[g] = wbf[a].reshape(P, CH * X)
        for g in range(len(slots[i]), G):  # pad slots: zero mask, any weight
            wt[g] = wt[0]
        ytm = np.ascontiguousarray(
            ytm.reshape(G, CH, P, BC).transpose(2, 0, 1, 3)
        ).reshape(P, G * CH * BC)
        in_maps.append(
            {
                "xt": xt,
                "ytm": ytm.astype(_np_bf16),
                "wt": wt,
                "biasg": np.ascontiguousarray(bias[actions[sel]]),
                "masku": x_mask[sel].astype(np.uint8),
                "eye16": eye,
            }
        )

    from concourse import bass_utils

    res = bass_utils.run_bass_kernel_spmd(
        nc, in_maps, core_ids=list(range(N_CORES))
    )
    _cached["last_results"] = res
    out_sorted = np.concatenate(
        [res.results[i]["out"] for i in range(N_CORES)], axis=0
    )
    out = np.empty_like(out_sorted)
    out[order] = out_sorted
    return out
